# revision 19
# baseline (speedup 1.0000x reference)
"""Trainium2 Bass kernel for the slimmable-conv MoE-routing module.

Reference computation (B=16, C=128, L=32768, G=4):
  pool   = mean(x, axis=-1)                      [B, C]
  logits = pool @ w_gate.T                       [B, G]
  gate   = straight-through gumbel softmax       [B, G]  (~one-hot)
  z      = conv_w @ x + conv_b                   [B, C, L]  (pointwise conv)
  out1   = z * (gate @ MASK)                     (channel gating)
  xn     = (out1 - gate@rmean) / sqrt(gate@rvar + eps) * bn_w + bn_b
  out    = xn * (gate @ MASK)

Everything after the pool reduces to a per-(batch,channel) affine applied to
the conv output:  out[b,c,l] = z_mm[b,c,l] * S[b,c] + T[b,c]  where z_mm is
the pure matmul part and S/T fold the gate, conv bias and BN constants.

Sharding: data-parallel over batch, 2 batches per core, 8 cores.  Per core:

  phase A: stream all 32 x-chunks, per-channel sums -> pool.  RES_PER_B
           chunks per batch are DMA'd straight into persistent SBUF tiles;
           the idle PE converts them to z = conv_w @ x in place.  The other
           chunks go through a small recycling pool.
  gate(b): tiny per-batch gating chain -> S[:,b], T[:,b]
  phase C: non-resident chunks: re-stream x, z = conv_w@x (PE), epilogue
           z*S+T (ACT/DVE), store.  Resident chunks: in-place z*S+T, store
           (pure writes, scheduled to drain at the tail).

HBM traffic per core: 64 + 2*(16 - RES_PER_B) MiB  (vs 96 naive).
"""

import numpy as np

import concourse.bass as bass
import concourse.tile as tile
from concourse import mybir, bacc
from concourse.bass_utils import run_bass_kernel_spmd
from concourse.masks import make_identity

F32 = mybir.dt.float32

B, C, L, G = 16, 128, 32768, 4
NCORES = 8
BPC = B // NCORES          # batches per core
CHANNELS = [32, 64, 96, 128]
EPS = 1e-5

LC = 2048                  # columns per DMA chunk
NCHUNK = L // LC           # chunks per batch
MMN = 512                  # matmul moving-dim (fp32 max)
NMM = LC // MMN            # matmuls per chunk
RES_PER_B = 8              # resident-z chunks per batch

AX = mybir.AxisListType.X
ALU = mybir.AluOpType
ACTF = mybir.ActivationFunctionType


def build_kernel(l_total=L, res_per_b=RES_PER_B):
    nchunk = l_total // LC
    res_per_b = min(res_per_b, nchunk)
    nc = bacc.Bacc("TRN2", target_bir_lowering=False)

    x_d = nc.declare_dram_parameter("x", [BPC, C, l_total], F32, isOutput=False)
    gum_d = nc.declare_dram_parameter("gumbel", [BPC, G], F32, isOutput=False)
    wg_d = nc.declare_dram_parameter("w_gate", [G, C], F32, isOutput=False)
    cw_d = nc.declare_dram_parameter("conv_w", [C, C], F32, isOutput=False)
    cb_d = nc.declare_dram_parameter("conv_b", [C, 1], F32, isOutput=False)
    bw_d = nc.declare_dram_parameter("bn_w", [C, 1], F32, isOutput=False)
    bb_d = nc.declare_dram_parameter("bn_b", [C, 1], F32, isOutput=False)
    rm_d = nc.declare_dram_parameter("rmean", [G, C], F32, isOutput=False)
    rv_d = nc.declare_dram_parameter("rvar", [G, C], F32, isOutput=False)
    out_d = nc.declare_dram_parameter("out", [BPC, C, l_total], F32, isOutput=True)

    # resident chunk ids: a contiguous prefix is fine; phase C emission order
    # handles read/write interleaving.
    res_set = set(range(res_per_b))

    def is_res(ci):
        return ci in res_set

    with tile.TileContext(nc) as tc:
        with (
            tc.tile_pool(name="consts", bufs=1) as consts,
            tc.tile_pool(name="xin", bufs=6) as xin_pool,
            tc.tile_pool(name="zres", bufs=1) as zres_pool,
            tc.tile_pool(name="small", bufs=1) as small,
            tc.tile_pool(name="scr", bufs=2) as scr_pool,
            tc.tile_pool(name="psz", bufs=4, space="PSUM") as psz,
            tc.tile_pool(name="pss", bufs=1, space="PSUM") as pss,
        ):
            # ---- constants ----
            id128 = consts.tile([128, 128], F32)
            make_identity(nc, id128)

            w_oi = consts.tile([C, C], F32)
            nc.sync.dma_start(out=w_oi, in_=cw_d.ap())
            wt_ps = pss.tile([C, C], F32, tag="big")
            nc.tensor.transpose(out=wt_ps, in_=w_oi, identity=id128)
            convwT = consts.tile([C, C], F32)       # [i, o] = conv_w[o, i]
            nc.vector.tensor_copy(out=convwT, in_=wt_ps)

            wgT = consts.tile([C, G], F32)          # [c, g] = w_gate[g, c]
            nc.sync.dma_start(out=wgT, in_=wg_d.ap().rearrange("g c -> c g"))
            gum_rows = []
            for b in range(BPC):
                gr = consts.tile([1, G], F32, tag=f"gum{b}")
                nc.sync.dma_start(out=gr, in_=gum_d.ap()[b:b + 1, :])
                gum_rows.append(gr)
            cb_sb = consts.tile([C, 1], F32)
            nc.sync.dma_start(out=cb_sb, in_=cb_d.ap())
            bw_sb = consts.tile([C, 1], F32)
            nc.sync.dma_start(out=bw_sb, in_=bw_d.ap())
            bb_sb = consts.tile([C, 1], F32)
            nc.sync.dma_start(out=bb_sb, in_=bb_d.ap())
            rm_sb = consts.tile([G, C], F32)
            nc.sync.dma_start(out=rm_sb, in_=rm_d.ap())
            rv_sb = consts.tile([G, C], F32)
            nc.sync.dma_start(out=rv_sb, in_=rv_d.ap())

            eps_sb = consts.tile([C, 1], F32)
            nc.vector.memset(eps_sb, EPS)

            # MASK[g, c] = 1.0 if c < CHANNELS[g] = 32*(g+1) else 0.0
            # iota = -32 - 32*g + c ; >= 0 -> keep in_ (0), else fill (1)
            mask_sb = consts.tile([G, C], F32)
            nc.gpsimd.memset(mask_sb, 0.0)
            nc.gpsimd.affine_select(
                out=mask_sb, in_=mask_sb, compare_op=ALU.is_ge, fill=1.0,
                base=-CHANNELS[0], channel_multiplier=-CHANNELS[0],
                pattern=[[1, C]])

            partials = consts.tile([C, 2 * BPC * nchunk], F32)
            pool_sb = consts.tile([C, BPC], F32)
            S_sb = consts.tile([C, BPC], F32)
            T_sb = consts.tile([C, BPC], F32)

            # ---- precompute S/T for each of the G possible gate choices ----
            # (forward gate is numerically the hard one-hot; see module doc)
            rmT = consts.tile([C, G], F32)          # [c, g] = rmean[g, c]
            nc.sync.dma_start(out=rmT, in_=rm_d.ap().rearrange("g c -> c g"))
            rvT = consts.tile([C, G], F32)
            nc.sync.dma_start(out=rvT, in_=rv_d.ap().rearrange("g c -> c g"))
            # maskT[c, g] = 1.0 if c < 32*(g+1) else 0.0
            maskT = consts.tile([C, G], F32)
            nc.gpsimd.memset(maskT, 0.0)
            nc.gpsimd.affine_select(
                out=maskT, in_=maskT, compare_op=ALU.is_ge, fill=1.0,
                base=-CHANNELS[0], channel_multiplier=1,
                pattern=[[-CHANNELS[0], G]])

            stdA = small.tile([C, G], F32)
            nc.scalar.activation(out=stdA, in_=rvT, func=ACTF.Sqrt,
                                 bias=eps_sb, scale=1.0)
            istdA = small.tile([C, G], F32)
            nc.vector.reciprocal(out=istdA, in_=stdA)
            # S_all = mask * istd * bn_w      (mask^2 == mask)
            S_all = small.tile([C, G], F32)
            nc.vector.tensor_mul(out=S_all, in0=maskT, in1=istdA)
            nc.vector.tensor_scalar_mul(out=S_all, in0=S_all, scalar1=bw_sb)
            # T_all = ((conv_b*mask - rmean) * istd * bn_w + bn_b) * mask
            T_all = small.tile([C, G], F32)
            nc.vector.tensor_scalar_mul(out=T_all, in0=maskT, scalar1=cb_sb)
            nc.vector.tensor_sub(out=T_all, in0=T_all, in1=rmT)
            nc.vector.tensor_mul(out=T_all, in0=T_all, in1=istdA)
            nc.vector.tensor_scalar(out=T_all, in0=T_all, scalar1=bw_sb,
                                    scalar2=bb_sb, op0=ALU.mult, op1=ALU.add)
            nc.vector.tensor_mul(out=T_all, in0=T_all, in1=maskT)
            # transpose to [G, C] for the one-hot selection matmuls
            sat_ps = pss.tile([G, C], F32, tag="big")
            nc.tensor.transpose(out=sat_ps, in_=S_all, identity=id128)
            S_allT = consts.tile([G, C], F32)
            nc.vector.tensor_copy(out=S_allT, in_=sat_ps)
            tat_ps = pss.tile([G, C], F32, tag="big")
            nc.tensor.transpose(out=tat_ps, in_=T_all, identity=id128)
            T_allT = consts.tile([G, C], F32)
            nc.vector.tensor_copy(out=T_allT, in_=tat_ps)

            zres = {}

            def phase_a_chunk(b, ci):
                col = b * nchunk + ci
                if is_res(ci):
                    dst = zres_pool.tile([C, LC], F32, tag=f"zres{col}")
                    zres[(b, ci)] = dst
                else:
                    dst = xin_pool.tile([C, LC], F32, tag="xin")
                nc.sync.dma_start(
                    out=dst, in_=x_d.ap()[b, :, ci * LC:(ci + 1) * LC])
                h = LC // 2
                nc.vector.reduce_sum(
                    out=partials[:, 2 * col:2 * col + 1], in_=dst[:, 0:h],
                    axis=AX)
                scr = scr_pool.tile([C, LC // 2], F32, tag="scr")
                nc.scalar.activation(
                    out=scr, in_=dst[:, h:LC], func=ACTF.Copy,
                    accum_out=partials[:, 2 * col + 1:2 * col + 2])
                if is_res(ci):
                    # convert x -> z in place (PE matmul + ACT drain)
                    for j in range(NMM):
                        js = slice(j * MMN, (j + 1) * MMN)
                        zp = psz.tile([C, MMN], F32)
                        nc.tensor.matmul(out=zp, lhsT=convwT, rhs=dst[:, js],
                                         start=True, stop=True)
                        nc.scalar.copy(out=dst[:, js], in_=zp)

            def finish_pool(b):
                nc.vector.reduce_sum(
                    out=pool_sb[:, b:b + 1],
                    in_=partials[:, 2 * b * nchunk:2 * (b + 1) * nchunk],
                    axis=AX)
                nc.scalar.mul(out=pool_sb[:, b:b + 1],
                              in_=pool_sb[:, b:b + 1], mul=1.0 / l_total)

            def gate_phase(b):
                """Short gating chain: logits -> hard one-hot -> select
                precomputed S/T columns via tiny matmuls."""
                lg_ps = pss.tile([1, G], F32, tag="lg")
                nc.tensor.matmul(out=lg_ps, lhsT=pool_sb[:, b:b + 1], rhs=wgT,
                                 start=True, stop=True)
                y_sb = small.tile([1, G], F32, tag=f"y{b}")
                nc.vector.tensor_add(out=y_sb, in0=lg_ps, in1=gum_rows[b])
                m1 = small.tile([1, 1], F32, tag=f"m1{b}")
                nc.vector.reduce_max(out=m1, in_=y_sb, axis=AX)
                yhard = small.tile([1, G], F32, tag=f"yh{b}")
                nc.vector.tensor_scalar(out=yhard, in0=y_sb, scalar1=m1,
                                        scalar2=None, op0=ALU.is_ge)
                gt_ps = pss.tile([G, 1], F32, tag="gt")
                nc.tensor.transpose(out=gt_ps, in_=yhard,
                                    identity=id128[0:1, 0:1])
                gateT = small.tile([G, 1], F32, tag=f"gT{b}")
                nc.vector.tensor_copy(out=gateT, in_=gt_ps)

                sel_ps = pss.tile([C, 2], F32, tag="big")
                nc.tensor.matmul(out=sel_ps[:, 0:1], lhsT=S_allT, rhs=gateT,
                                 start=True, stop=True)
                nc.tensor.matmul(out=sel_ps[:, 1:2], lhsT=T_allT, rhs=gateT,
                                 start=True, stop=True)
                nc.vector.tensor_copy(out=S_sb[:, b:b + 1], in_=sel_ps[:, 0:1])
                nc.vector.tensor_copy(out=T_sb[:, b:b + 1], in_=sel_ps[:, 1:2])

            def c_chunk(b, ci, epi_parity):
                S_col = S_sb[:, b:b + 1]
                T_col = T_sb[:, b:b + 1]
                sl = slice(ci * LC, (ci + 1) * LC)
                if is_res(ci):
                    zt = zres[(b, ci)]
                    nc.vector.tensor_scalar(
                        out=zt, in0=zt, scalar1=S_col, scalar2=T_col,
                        op0=ALU.mult, op1=ALU.add)
                    nc.sync.dma_start(out=out_d.ap()[b, :, sl], in_=zt)
                    return
                xc = xin_pool.tile([C, LC], F32, tag="xin")
                nc.sync.dma_start(out=xc, in_=x_d.ap()[b, :, sl])
                for j in range(NMM):
                    js = slice(j * MMN, (j + 1) * MMN)
                    zp = psz.tile([C, MMN], F32)
                    nc.tensor.matmul(out=zp, lhsT=convwT, rhs=xc[:, js],
                                     start=True, stop=True)
                    if (epi_parity * NMM + j) % 2 == 0:
                        nc.scalar.activation(out=xc[:, js], in_=zp,
                                             func=ACTF.Identity,
                                             bias=T_col, scale=S_col)
                    else:
                        nc.vector.tensor_scalar(
                            out=xc[:, js], in0=zp, scalar1=S_col,
                            scalar2=T_col, op0=ALU.mult, op1=ALU.add)
                nc.sync.dma_start(out=out_d.ap()[b, :, sl], in_=xc)

            # ---- emission order ----
            with nc.named_scope("phaseA"):
                for b in range(BPC):
                    for ci in range(nchunk):
                        phase_a_chunk(b, ci)
                    finish_pool(b)
            with nc.named_scope("gates"):
                for b in range(BPC):
                    gate_phase(b)
            # phase C: non-resident first (reads can prefetch during the
            # gating chain), resident last (pure writes drain the tail).
            with nc.named_scope("phaseC"):
                parity = 0
                for b in range(BPC):
                    for ci in range(nchunk):
                        if not is_res(ci):
                            c_chunk(b, ci, parity)
                            parity += 1
                for b in range(BPC):
                    for ci in range(nchunk):
                        if is_res(ci):
                            c_chunk(b, ci, 0)

    nc.compile()
    return nc


_NC = None


def _get_nc():
    global _NC
    if _NC is None:
        _NC = build_kernel()
    return _NC


def kernel(x, gumbel_noise, w_gate, conv_w, conv_b, bn_w, bn_b, rmean, rvar):
    nc = _get_nc()
    f = lambda a: np.ascontiguousarray(a, dtype=np.float32)
    shared = {
        "w_gate": f(w_gate),
        "conv_w": f(conv_w),
        "conv_b": f(conv_b).reshape(C, 1),
        "bn_w": f(bn_w).reshape(C, 1),
        "bn_b": f(bn_b).reshape(C, 1),
        "rmean": f(rmean),
        "rvar": f(rvar),
    }
    in_maps = []
    for i in range(NCORES):
        sl = slice(i * BPC, (i + 1) * BPC)
        in_maps.append({"x": f(x[sl]), "gumbel": f(gumbel_noise[sl]), **shared})
    res = run_bass_kernel_spmd(nc, in_maps, list(range(NCORES)))
    out = np.concatenate([res.results[i]["out"] for i in range(NCORES)], axis=0)
    return out.astype(np.float32, copy=False)


# revision 20
# speedup vs baseline: 1.0177x; 1.0177x over previous
"""Trainium2 Bass kernel for the slimmable-conv MoE-routing module.

Reference computation (B=16, C=128, L=32768, G=4):
  pool   = mean(x, axis=-1)                      [B, C]
  logits = pool @ w_gate.T                       [B, G]
  gate   = straight-through gumbel softmax       [B, G]  (~one-hot)
  z      = conv_w @ x + conv_b                   [B, C, L]  (pointwise conv)
  out1   = z * (gate @ MASK)                     (channel gating)
  xn     = (out1 - gate@rmean) / sqrt(gate@rvar + eps) * bn_w + bn_b
  out    = xn * (gate @ MASK)

Everything after the pool reduces to a per-(batch,channel) affine applied to
the conv output:  out[b,c,l] = z_mm[b,c,l] * S[b,c] + T[b,c]  where z_mm is
the pure matmul part and S/T fold the gate, conv bias and BN constants.

Sharding: data-parallel over batch, 2 batches per core, 8 cores.  Per core:

  phase A: stream all 32 x-chunks, per-channel sums -> pool.  RES_PER_B
           chunks per batch are DMA'd straight into persistent SBUF tiles;
           the idle PE converts them to z = conv_w @ x in place.  The other
           chunks go through a small recycling pool.
  gate(b): tiny per-batch gating chain -> S[:,b], T[:,b]
  phase C: non-resident chunks: re-stream x, z = conv_w@x (PE), epilogue
           z*S+T (ACT/DVE), store.  Resident chunks: in-place z*S+T, store
           (pure writes, scheduled to drain at the tail).

HBM traffic per core: 64 + 2*(16 - RES_PER_B) MiB  (vs 96 naive).
"""

import numpy as np

import concourse.bass as bass
import concourse.tile as tile
from concourse import mybir, bacc
from concourse.bass_utils import run_bass_kernel_spmd
from concourse.masks import make_identity

F32 = mybir.dt.float32

B, C, L, G = 16, 128, 32768, 4
NCORES = 8
BPC = B // NCORES          # batches per core
CHANNELS = [32, 64, 96, 128]
EPS = 1e-5

LC = 2048                  # columns per DMA chunk
NCHUNK = L // LC           # chunks per batch
MMN = 512                  # matmul moving-dim (fp32 max)
NMM = LC // MMN            # matmuls per chunk
RES_PER_B = 9              # resident-z chunks per batch

AX = mybir.AxisListType.X
ALU = mybir.AluOpType
ACTF = mybir.ActivationFunctionType


def build_kernel(l_total=L, res_per_b=RES_PER_B):
    nchunk = l_total // LC
    res_per_b = min(res_per_b, nchunk)
    nc = bacc.Bacc("TRN2", target_bir_lowering=False)

    x_d = nc.declare_dram_parameter("x", [BPC, C, l_total], F32, isOutput=False)
    gum_d = nc.declare_dram_parameter("gumbel", [BPC, G], F32, isOutput=False)
    wg_d = nc.declare_dram_parameter("w_gate", [G, C], F32, isOutput=False)
    cw_d = nc.declare_dram_parameter("conv_w", [C, C], F32, isOutput=False)
    cb_d = nc.declare_dram_parameter("conv_b", [C, 1], F32, isOutput=False)
    bw_d = nc.declare_dram_parameter("bn_w", [C, 1], F32, isOutput=False)
    bb_d = nc.declare_dram_parameter("bn_b", [C, 1], F32, isOutput=False)
    rm_d = nc.declare_dram_parameter("rmean", [G, C], F32, isOutput=False)
    rv_d = nc.declare_dram_parameter("rvar", [G, C], F32, isOutput=False)
    out_d = nc.declare_dram_parameter("out", [BPC, C, l_total], F32, isOutput=True)

    # resident chunk ids: a contiguous prefix is fine; phase C emission order
    # handles read/write interleaving.
    res_set = set(range(res_per_b))

    def is_res(ci):
        return ci in res_set

    with tile.TileContext(nc) as tc:
        with (
            tc.tile_pool(name="consts", bufs=1) as consts,
            tc.tile_pool(name="xin", bufs=5) as xin_pool,
            tc.tile_pool(name="zres", bufs=1) as zres_pool,
            tc.tile_pool(name="small", bufs=1) as small,
            tc.tile_pool(name="psz", bufs=4, space="PSUM") as psz,
            tc.tile_pool(name="pss", bufs=1, space="PSUM") as pss,
        ):
            # ---- constants ----
            id128 = consts.tile([128, 128], F32)
            make_identity(nc, id128)

            w_oi = consts.tile([C, C], F32)
            nc.sync.dma_start(out=w_oi, in_=cw_d.ap())
            wt_ps = pss.tile([C, C], F32, tag="big")
            nc.tensor.transpose(out=wt_ps, in_=w_oi, identity=id128)
            convwT = consts.tile([C, C], F32)       # [i, o] = conv_w[o, i]
            nc.vector.tensor_copy(out=convwT, in_=wt_ps)

            wgT = consts.tile([C, G], F32)          # [c, g] = w_gate[g, c]
            nc.sync.dma_start(out=wgT, in_=wg_d.ap().rearrange("g c -> c g"))
            gum_rows = []
            for b in range(BPC):
                gr = consts.tile([1, G], F32, tag=f"gum{b}")
                nc.sync.dma_start(out=gr, in_=gum_d.ap()[b:b + 1, :])
                gum_rows.append(gr)
            cb_sb = consts.tile([C, 1], F32)
            nc.sync.dma_start(out=cb_sb, in_=cb_d.ap())
            bw_sb = consts.tile([C, 1], F32)
            nc.sync.dma_start(out=bw_sb, in_=bw_d.ap())
            bb_sb = consts.tile([C, 1], F32)
            nc.sync.dma_start(out=bb_sb, in_=bb_d.ap())
            rm_sb = consts.tile([G, C], F32)
            nc.sync.dma_start(out=rm_sb, in_=rm_d.ap())
            rv_sb = consts.tile([G, C], F32)
            nc.sync.dma_start(out=rv_sb, in_=rv_d.ap())

            eps_sb = consts.tile([C, 1], F32)
            nc.vector.memset(eps_sb, EPS)

            # MASK[g, c] = 1.0 if c < CHANNELS[g] = 32*(g+1) else 0.0
            # iota = -32 - 32*g + c ; >= 0 -> keep in_ (0), else fill (1)
            mask_sb = consts.tile([G, C], F32)
            nc.gpsimd.memset(mask_sb, 0.0)
            nc.gpsimd.affine_select(
                out=mask_sb, in_=mask_sb, compare_op=ALU.is_ge, fill=1.0,
                base=-CHANNELS[0], channel_multiplier=-CHANNELS[0],
                pattern=[[1, C]])

            partials = consts.tile([C, 2 * BPC * nchunk], F32)
            pool_sb = consts.tile([C, BPC], F32)
            S_sb = consts.tile([C, BPC], F32)
            T_sb = consts.tile([C, BPC], F32)

            # ---- precompute S/T for each of the G possible gate choices ----
            # (forward gate is numerically the hard one-hot; see module doc)
            rmT = consts.tile([C, G], F32)          # [c, g] = rmean[g, c]
            nc.sync.dma_start(out=rmT, in_=rm_d.ap().rearrange("g c -> c g"))
            rvT = consts.tile([C, G], F32)
            nc.sync.dma_start(out=rvT, in_=rv_d.ap().rearrange("g c -> c g"))
            # maskT[c, g] = 1.0 if c < 32*(g+1) else 0.0
            maskT = consts.tile([C, G], F32)
            nc.gpsimd.memset(maskT, 0.0)
            nc.gpsimd.affine_select(
                out=maskT, in_=maskT, compare_op=ALU.is_ge, fill=1.0,
                base=-CHANNELS[0], channel_multiplier=1,
                pattern=[[-CHANNELS[0], G]])

            stdA = small.tile([C, G], F32)
            nc.scalar.activation(out=stdA, in_=rvT, func=ACTF.Sqrt,
                                 bias=eps_sb, scale=1.0)
            istdA = small.tile([C, G], F32)
            nc.vector.reciprocal(out=istdA, in_=stdA)
            # S_all = mask * istd * bn_w      (mask^2 == mask)
            S_all = small.tile([C, G], F32)
            nc.vector.tensor_mul(out=S_all, in0=maskT, in1=istdA)
            nc.vector.tensor_scalar_mul(out=S_all, in0=S_all, scalar1=bw_sb)
            # T_all = ((conv_b*mask - rmean) * istd * bn_w + bn_b) * mask
            T_all = small.tile([C, G], F32)
            nc.vector.tensor_scalar_mul(out=T_all, in0=maskT, scalar1=cb_sb)
            nc.vector.tensor_sub(out=T_all, in0=T_all, in1=rmT)
            nc.vector.tensor_mul(out=T_all, in0=T_all, in1=istdA)
            nc.vector.tensor_scalar(out=T_all, in0=T_all, scalar1=bw_sb,
                                    scalar2=bb_sb, op0=ALU.mult, op1=ALU.add)
            nc.vector.tensor_mul(out=T_all, in0=T_all, in1=maskT)
            # transpose to [G, C] for the one-hot selection matmuls
            sat_ps = pss.tile([G, C], F32, tag="big")
            nc.tensor.transpose(out=sat_ps, in_=S_all, identity=id128)
            S_allT = consts.tile([G, C], F32)
            nc.vector.tensor_copy(out=S_allT, in_=sat_ps)
            tat_ps = pss.tile([G, C], F32, tag="big")
            nc.tensor.transpose(out=tat_ps, in_=T_all, identity=id128)
            T_allT = consts.tile([G, C], F32)
            nc.vector.tensor_copy(out=T_allT, in_=tat_ps)

            zres = {}

            def phase_a_chunk(b, ci):
                col = b * nchunk + ci
                if is_res(ci):
                    dst = zres_pool.tile([C, LC], F32, tag=f"zres{col}")
                    zres[(b, ci)] = dst
                else:
                    dst = xin_pool.tile([C, LC], F32, tag="xin")
                nc.sync.dma_start(
                    out=dst, in_=x_d.ap()[b, :, ci * LC:(ci + 1) * LC])
                h = LC // 2
                nc.vector.reduce_sum(
                    out=partials[:, 2 * col:2 * col + 1], in_=dst[:, 0:h],
                    axis=AX)
                nc.scalar.activation(
                    out=dst[:, h:LC], in_=dst[:, h:LC], func=ACTF.Copy,
                    accum_out=partials[:, 2 * col + 1:2 * col + 2])
                if is_res(ci):
                    # convert x -> z in place (PE matmul + ACT drain)
                    for j in range(NMM):
                        js = slice(j * MMN, (j + 1) * MMN)
                        zp = psz.tile([C, MMN], F32)
                        nc.tensor.matmul(out=zp, lhsT=convwT, rhs=dst[:, js],
                                         start=True, stop=True)
                        nc.scalar.copy(out=dst[:, js], in_=zp)

            def finish_pool(b):
                nc.vector.reduce_sum(
                    out=pool_sb[:, b:b + 1],
                    in_=partials[:, 2 * b * nchunk:2 * (b + 1) * nchunk],
                    axis=AX)
                nc.scalar.mul(out=pool_sb[:, b:b + 1],
                              in_=pool_sb[:, b:b + 1], mul=1.0 / l_total)

            def gate_phase(b):
                """Short gating chain: logits -> hard one-hot -> select
                precomputed S/T columns via tiny matmuls."""
                lg_ps = pss.tile([1, G], F32, tag="lg")
                nc.tensor.matmul(out=lg_ps, lhsT=pool_sb[:, b:b + 1], rhs=wgT,
                                 start=True, stop=True)
                y_sb = small.tile([1, G], F32, tag=f"y{b}")
                nc.vector.tensor_add(out=y_sb, in0=lg_ps, in1=gum_rows[b])
                m1 = small.tile([1, 1], F32, tag=f"m1{b}")
                nc.vector.reduce_max(out=m1, in_=y_sb, axis=AX)
                yhard = small.tile([1, G], F32, tag=f"yh{b}")
                nc.vector.tensor_scalar(out=yhard, in0=y_sb, scalar1=m1,
                                        scalar2=None, op0=ALU.is_ge)
                gt_ps = pss.tile([G, 1], F32, tag="gt")
                nc.tensor.transpose(out=gt_ps, in_=yhard,
                                    identity=id128[0:1, 0:1])
                gateT = small.tile([G, 1], F32, tag=f"gT{b}")
                nc.vector.tensor_copy(out=gateT, in_=gt_ps)

                sel_ps = pss.tile([C, 2], F32, tag="big")
                nc.tensor.matmul(out=sel_ps[:, 0:1], lhsT=S_allT, rhs=gateT,
                                 start=True, stop=True)
                nc.tensor.matmul(out=sel_ps[:, 1:2], lhsT=T_allT, rhs=gateT,
                                 start=True, stop=True)
                nc.vector.tensor_copy(out=S_sb[:, b:b + 1], in_=sel_ps[:, 0:1])
                nc.vector.tensor_copy(out=T_sb[:, b:b + 1], in_=sel_ps[:, 1:2])

            def c_chunk(b, ci, epi_parity):
                S_col = S_sb[:, b:b + 1]
                T_col = T_sb[:, b:b + 1]
                sl = slice(ci * LC, (ci + 1) * LC)
                if is_res(ci):
                    zt = zres[(b, ci)]
                    nc.vector.tensor_scalar(
                        out=zt, in0=zt, scalar1=S_col, scalar2=T_col,
                        op0=ALU.mult, op1=ALU.add)
                    nc.sync.dma_start(out=out_d.ap()[b, :, sl], in_=zt)
                    return
                xc = xin_pool.tile([C, LC], F32, tag="xin")
                nc.sync.dma_start(out=xc, in_=x_d.ap()[b, :, sl])
                for j in range(NMM):
                    js = slice(j * MMN, (j + 1) * MMN)
                    zp = psz.tile([C, MMN], F32)
                    nc.tensor.matmul(out=zp, lhsT=convwT, rhs=xc[:, js],
                                     start=True, stop=True)
                    if (epi_parity * NMM + j) % 2 == 0:
                        nc.scalar.activation(out=xc[:, js], in_=zp,
                                             func=ACTF.Identity,
                                             bias=T_col, scale=S_col)
                    else:
                        nc.vector.tensor_scalar(
                            out=xc[:, js], in0=zp, scalar1=S_col,
                            scalar2=T_col, op0=ALU.mult, op1=ALU.add)
                nc.sync.dma_start(out=out_d.ap()[b, :, sl], in_=xc)

            # ---- emission order ----
            with nc.named_scope("phaseA"):
                for b in range(BPC):
                    for ci in range(nchunk):
                        phase_a_chunk(b, ci)
                    finish_pool(b)
            with nc.named_scope("gates"):
                for b in range(BPC):
                    gate_phase(b)
            # phase C: non-resident first (reads can prefetch during the
            # gating chain), resident last (pure writes drain the tail).
            with nc.named_scope("phaseC"):
                parity = 0
                for b in range(BPC):
                    for ci in range(nchunk):
                        if not is_res(ci):
                            c_chunk(b, ci, parity)
                            parity += 1
                for b in range(BPC):
                    for ci in range(nchunk):
                        if is_res(ci):
                            c_chunk(b, ci, 0)

    nc.compile()
    return nc


_NC = None


def _get_nc():
    global _NC
    if _NC is None:
        _NC = build_kernel()
    return _NC


def kernel(x, gumbel_noise, w_gate, conv_w, conv_b, bn_w, bn_b, rmean, rvar):
    nc = _get_nc()
    f = lambda a: np.ascontiguousarray(a, dtype=np.float32)
    shared = {
        "w_gate": f(w_gate),
        "conv_w": f(conv_w),
        "conv_b": f(conv_b).reshape(C, 1),
        "bn_w": f(bn_w).reshape(C, 1),
        "bn_b": f(bn_b).reshape(C, 1),
        "rmean": f(rmean),
        "rvar": f(rvar),
    }
    in_maps = []
    for i in range(NCORES):
        sl = slice(i * BPC, (i + 1) * BPC)
        in_maps.append({"x": f(x[sl]), "gumbel": f(gumbel_noise[sl]), **shared})
    res = run_bass_kernel_spmd(nc, in_maps, list(range(NCORES)))
    out = np.concatenate([res.results[i]["out"] for i in range(NCORES)], axis=0)
    return out.astype(np.float32, copy=False)


# revision 21
# speedup vs baseline: 1.0264x; 1.0086x over previous
"""Trainium2 Bass kernel for the slimmable-conv MoE-routing module.

Reference computation (B=16, C=128, L=32768, G=4):
  pool   = mean(x, axis=-1)                      [B, C]
  logits = pool @ w_gate.T                       [B, G]
  gate   = straight-through gumbel softmax       [B, G]  (~one-hot)
  z      = conv_w @ x + conv_b                   [B, C, L]  (pointwise conv)
  out1   = z * (gate @ MASK)                     (channel gating)
  xn     = (out1 - gate@rmean) / sqrt(gate@rvar + eps) * bn_w + bn_b
  out    = xn * (gate @ MASK)

Everything after the pool reduces to a per-(batch,channel) affine applied to
the conv output:  out[b,c,l] = z_mm[b,c,l] * S[b,c] + T[b,c]  where z_mm is
the pure matmul part and S/T fold the gate, conv bias and BN constants.

Sharding: data-parallel over batch, 2 batches per core, 8 cores.  Per core:

  phase A: stream all 32 x-chunks, per-channel sums -> pool.  RES_PER_B
           chunks per batch are DMA'd straight into persistent SBUF tiles;
           the idle PE converts them to z = conv_w @ x in place.  The other
           chunks go through a small recycling pool.
  gate(b): tiny per-batch gating chain -> S[:,b], T[:,b]
  phase C: non-resident chunks: re-stream x, z = conv_w@x (PE), epilogue
           z*S+T (ACT/DVE), store.  Resident chunks: in-place z*S+T, store
           (pure writes, scheduled to drain at the tail).

HBM traffic per core: 64 + 2*(16 - RES_PER_B) MiB  (vs 96 naive).
"""

import numpy as np

import concourse.bass as bass
import concourse.tile as tile
from concourse import mybir, bacc
from concourse.bass_utils import run_bass_kernel_spmd
from concourse.masks import make_identity

F32 = mybir.dt.float32

B, C, L, G = 16, 128, 32768, 4
NCORES = 8
BPC = B // NCORES          # batches per core
CHANNELS = [32, 64, 96, 128]
EPS = 1e-5

LC = 2048                  # columns per DMA chunk
NCHUNK = L // LC           # chunks per batch
MMN = 512                  # matmul moving-dim (fp32 max)
NMM = LC // MMN            # matmuls per chunk
N_RES = 17                 # total resident-z chunks (of BPC*NCHUNK)

AX = mybir.AxisListType.X
ALU = mybir.AluOpType
ACTF = mybir.ActivationFunctionType


def build_kernel(l_total=L, n_res=N_RES):
    nchunk = l_total // LC
    n_res = min(n_res, BPC * nchunk)
    nc = bacc.Bacc("TRN2", target_bir_lowering=False)

    x_d = nc.declare_dram_parameter("x", [BPC, C, l_total], F32, isOutput=False)
    gum_d = nc.declare_dram_parameter("gumbel", [BPC, G], F32, isOutput=False)
    wg_d = nc.declare_dram_parameter("w_gate", [G, C], F32, isOutput=False)
    cw_d = nc.declare_dram_parameter("conv_w", [C, C], F32, isOutput=False)
    cb_d = nc.declare_dram_parameter("conv_b", [C, 1], F32, isOutput=False)
    bw_d = nc.declare_dram_parameter("bn_w", [C, 1], F32, isOutput=False)
    bb_d = nc.declare_dram_parameter("bn_b", [C, 1], F32, isOutput=False)
    rm_d = nc.declare_dram_parameter("rmean", [G, C], F32, isOutput=False)
    rv_d = nc.declare_dram_parameter("rvar", [G, C], F32, isOutput=False)
    out_d = nc.declare_dram_parameter("out", [BPC, C, l_total], F32, isOutput=True)

    def is_res(b, ci):
        return b * nchunk + ci < n_res

    with tile.TileContext(nc) as tc:
        with (
            tc.tile_pool(name="consts", bufs=1) as consts,
            tc.tile_pool(name="xin", bufs=6) as xin_pool,
            tc.tile_pool(name="zres", bufs=1) as zres_pool,
            tc.tile_pool(name="small", bufs=1) as small,
            tc.tile_pool(name="psz", bufs=4, space="PSUM") as psz,
            tc.tile_pool(name="pss", bufs=1, space="PSUM") as pss,
        ):
            # ---- constants ----
            id128 = consts.tile([128, 128], F32)
            make_identity(nc, id128)

            w_oi = consts.tile([C, C], F32)
            nc.sync.dma_start(out=w_oi, in_=cw_d.ap())
            wt_ps = pss.tile([C, C], F32, tag="big")
            nc.tensor.transpose(out=wt_ps, in_=w_oi, identity=id128)
            convwT = consts.tile([C, C], F32)       # [i, o] = conv_w[o, i]
            nc.vector.tensor_copy(out=convwT, in_=wt_ps)

            wgT = consts.tile([C, G], F32)          # [c, g] = w_gate[g, c]
            nc.sync.dma_start(out=wgT, in_=wg_d.ap().rearrange("g c -> c g"))
            gum_rows = []
            for b in range(BPC):
                gr = consts.tile([1, G], F32, tag=f"gum{b}")
                nc.sync.dma_start(out=gr, in_=gum_d.ap()[b:b + 1, :])
                gum_rows.append(gr)
            cb_sb = consts.tile([C, 1], F32)
            nc.sync.dma_start(out=cb_sb, in_=cb_d.ap())
            bw_sb = consts.tile([C, 1], F32)
            nc.sync.dma_start(out=bw_sb, in_=bw_d.ap())
            bb_sb = consts.tile([C, 1], F32)
            nc.sync.dma_start(out=bb_sb, in_=bb_d.ap())
            rm_sb = consts.tile([G, C], F32)
            nc.sync.dma_start(out=rm_sb, in_=rm_d.ap())
            rv_sb = consts.tile([G, C], F32)
            nc.sync.dma_start(out=rv_sb, in_=rv_d.ap())

            eps_sb = consts.tile([C, 1], F32)
            nc.vector.memset(eps_sb, EPS)

            # MASK[g, c] = 1.0 if c < CHANNELS[g] = 32*(g+1) else 0.0
            # iota = -32 - 32*g + c ; >= 0 -> keep in_ (0), else fill (1)
            mask_sb = consts.tile([G, C], F32)
            nc.gpsimd.memset(mask_sb, 0.0)
            nc.gpsimd.affine_select(
                out=mask_sb, in_=mask_sb, compare_op=ALU.is_ge, fill=1.0,
                base=-CHANNELS[0], channel_multiplier=-CHANNELS[0],
                pattern=[[1, C]])

            partials = consts.tile([C, BPC * nchunk], F32)
            pool_sb = consts.tile([C, BPC], F32)
            S_sb = consts.tile([C, BPC], F32)
            T_sb = consts.tile([C, BPC], F32)

            # ---- precompute S/T for each of the G possible gate choices ----
            # (forward gate is numerically the hard one-hot; see module doc)
            rmT = consts.tile([C, G], F32)          # [c, g] = rmean[g, c]
            nc.sync.dma_start(out=rmT, in_=rm_d.ap().rearrange("g c -> c g"))
            rvT = consts.tile([C, G], F32)
            nc.sync.dma_start(out=rvT, in_=rv_d.ap().rearrange("g c -> c g"))
            # maskT[c, g] = 1.0 if c < 32*(g+1) else 0.0
            maskT = consts.tile([C, G], F32)
            nc.gpsimd.memset(maskT, 0.0)
            nc.gpsimd.affine_select(
                out=maskT, in_=maskT, compare_op=ALU.is_ge, fill=1.0,
                base=-CHANNELS[0], channel_multiplier=1,
                pattern=[[-CHANNELS[0], G]])

            stdA = small.tile([C, G], F32)
            nc.scalar.activation(out=stdA, in_=rvT, func=ACTF.Sqrt,
                                 bias=eps_sb, scale=1.0)
            istdA = small.tile([C, G], F32)
            nc.vector.reciprocal(out=istdA, in_=stdA)
            # S_all = mask * istd * bn_w      (mask^2 == mask)
            S_all = small.tile([C, G], F32)
            nc.vector.tensor_mul(out=S_all, in0=maskT, in1=istdA)
            nc.vector.tensor_scalar_mul(out=S_all, in0=S_all, scalar1=bw_sb)
            # T_all = ((conv_b*mask - rmean) * istd * bn_w + bn_b) * mask
            T_all = small.tile([C, G], F32)
            nc.vector.tensor_scalar_mul(out=T_all, in0=maskT, scalar1=cb_sb)
            nc.vector.tensor_sub(out=T_all, in0=T_all, in1=rmT)
            nc.vector.tensor_mul(out=T_all, in0=T_all, in1=istdA)
            nc.vector.tensor_scalar(out=T_all, in0=T_all, scalar1=bw_sb,
                                    scalar2=bb_sb, op0=ALU.mult, op1=ALU.add)
            nc.vector.tensor_mul(out=T_all, in0=T_all, in1=maskT)
            # transpose to [G, C] for the one-hot selection matmuls
            sat_ps = pss.tile([G, C], F32, tag="big")
            nc.tensor.transpose(out=sat_ps, in_=S_all, identity=id128)
            S_allT = consts.tile([G, C], F32)
            nc.vector.tensor_copy(out=S_allT, in_=sat_ps)
            tat_ps = pss.tile([G, C], F32, tag="big")
            nc.tensor.transpose(out=tat_ps, in_=T_all, identity=id128)
            T_allT = consts.tile([G, C], F32)
            nc.vector.tensor_copy(out=T_allT, in_=tat_ps)

            zres = {}

            def phase_a_chunk(b, ci):
                col = b * nchunk + ci
                if is_res(b, ci):
                    dst = zres_pool.tile([C, LC], F32, tag=f"zres{col}")
                    zres[(b, ci)] = dst
                else:
                    dst = xin_pool.tile([C, LC], F32, tag="xin")
                nc.sync.dma_start(
                    out=dst, in_=x_d.ap()[b, :, ci * LC:(ci + 1) * LC])
                nc.vector.reduce_sum(
                    out=partials[:, col:col + 1], in_=dst, axis=AX)
                if is_res(b, ci):
                    # convert x -> z in place (PE matmul + ACT drain)
                    for j in range(NMM):
                        js = slice(j * MMN, (j + 1) * MMN)
                        zp = psz.tile([C, MMN], F32)
                        nc.tensor.matmul(out=zp, lhsT=convwT, rhs=dst[:, js],
                                         start=True, stop=True)
                        nc.scalar.copy(out=dst[:, js], in_=zp)

            def finish_pool(b):
                nc.vector.reduce_sum(
                    out=pool_sb[:, b:b + 1],
                    in_=partials[:, b * nchunk:(b + 1) * nchunk], axis=AX)
                nc.scalar.mul(out=pool_sb[:, b:b + 1],
                              in_=pool_sb[:, b:b + 1], mul=1.0 / l_total)

            def gate_phase(b):
                """Short gating chain: logits -> hard one-hot -> select
                precomputed S/T columns via tiny matmuls."""
                lg_ps = pss.tile([1, G], F32, tag="lg")
                nc.tensor.matmul(out=lg_ps, lhsT=pool_sb[:, b:b + 1], rhs=wgT,
                                 start=True, stop=True)
                y_sb = small.tile([1, G], F32, tag=f"y{b}")
                nc.vector.tensor_add(out=y_sb, in0=lg_ps, in1=gum_rows[b])
                m1 = small.tile([1, 1], F32, tag=f"m1{b}")
                nc.vector.reduce_max(out=m1, in_=y_sb, axis=AX)
                yhard = small.tile([1, G], F32, tag=f"yh{b}")
                nc.vector.tensor_scalar(out=yhard, in0=y_sb, scalar1=m1,
                                        scalar2=None, op0=ALU.is_ge)
                gt_ps = pss.tile([G, 1], F32, tag="gt")
                nc.tensor.transpose(out=gt_ps, in_=yhard,
                                    identity=id128[0:1, 0:1])
                gateT = small.tile([G, 1], F32, tag=f"gT{b}")
                nc.vector.tensor_copy(out=gateT, in_=gt_ps)

                sel_ps = pss.tile([C, 2], F32, tag="big")
                nc.tensor.matmul(out=sel_ps[:, 0:1], lhsT=S_allT, rhs=gateT,
                                 start=True, stop=True)
                nc.tensor.matmul(out=sel_ps[:, 1:2], lhsT=T_allT, rhs=gateT,
                                 start=True, stop=True)
                nc.vector.tensor_copy(out=S_sb[:, b:b + 1], in_=sel_ps[:, 0:1])
                nc.vector.tensor_copy(out=T_sb[:, b:b + 1], in_=sel_ps[:, 1:2])

            def c_chunk(b, ci, epi_parity):
                S_col = S_sb[:, b:b + 1]
                T_col = T_sb[:, b:b + 1]
                sl = slice(ci * LC, (ci + 1) * LC)
                if is_res(b, ci):
                    zt = zres[(b, ci)]
                    nc.vector.tensor_scalar(
                        out=zt, in0=zt, scalar1=S_col, scalar2=T_col,
                        op0=ALU.mult, op1=ALU.add)
                    nc.sync.dma_start(out=out_d.ap()[b, :, sl], in_=zt)
                    return
                xc = xin_pool.tile([C, LC], F32, tag="xin")
                nc.sync.dma_start(out=xc, in_=x_d.ap()[b, :, sl])
                for j in range(NMM):
                    js = slice(j * MMN, (j + 1) * MMN)
                    zp = psz.tile([C, MMN], F32)
                    nc.tensor.matmul(out=zp, lhsT=convwT, rhs=xc[:, js],
                                     start=True, stop=True)
                    if (epi_parity * NMM + j) % 2 == 0:
                        nc.scalar.activation(out=xc[:, js], in_=zp,
                                             func=ACTF.Identity,
                                             bias=T_col, scale=S_col)
                    else:
                        nc.vector.tensor_scalar(
                            out=xc[:, js], in0=zp, scalar1=S_col,
                            scalar2=T_col, op0=ALU.mult, op1=ALU.add)
                nc.sync.dma_start(out=out_d.ap()[b, :, sl], in_=xc)

            # ---- emission order ----
            with nc.named_scope("phaseA"):
                for b in range(BPC):
                    for ci in range(nchunk):
                        phase_a_chunk(b, ci)
                    finish_pool(b)
            with nc.named_scope("gates"):
                for b in range(BPC):
                    gate_phase(b)
            # phase C: non-resident first (reads can prefetch during the
            # gating chain), resident last (pure writes drain the tail).
            with nc.named_scope("phaseC"):
                parity = 0
                for b in range(BPC):
                    for ci in range(nchunk):
                        if not is_res(b, ci):
                            c_chunk(b, ci, parity)
                            parity += 1
                for b in range(BPC):
                    for ci in range(nchunk):
                        if is_res(b, ci):
                            c_chunk(b, ci, 0)

    nc.compile()
    return nc


_NC = None


def _get_nc():
    global _NC
    if _NC is None:
        _NC = build_kernel()
    return _NC


def kernel(x, gumbel_noise, w_gate, conv_w, conv_b, bn_w, bn_b, rmean, rvar):
    nc = _get_nc()
    f = lambda a: np.ascontiguousarray(a, dtype=np.float32)
    shared = {
        "w_gate": f(w_gate),
        "conv_w": f(conv_w),
        "conv_b": f(conv_b).reshape(C, 1),
        "bn_w": f(bn_w).reshape(C, 1),
        "bn_b": f(bn_b).reshape(C, 1),
        "rmean": f(rmean),
        "rvar": f(rvar),
    }
    in_maps = []
    for i in range(NCORES):
        sl = slice(i * BPC, (i + 1) * BPC)
        in_maps.append({"x": f(x[sl]), "gumbel": f(gumbel_noise[sl]), **shared})
    res = run_bass_kernel_spmd(nc, in_maps, list(range(NCORES)))
    out = np.concatenate([res.results[i]["out"] for i in range(NCORES)], axis=0)
    return out.astype(np.float32, copy=False)


# revision 22
# speedup vs baseline: 1.0443x; 1.0174x over previous
"""Trainium2 Bass kernel for the slimmable-conv MoE-routing module.

Reference computation (B=16, C=128, L=32768, G=4):
  pool   = mean(x, axis=-1)                      [B, C]
  logits = pool @ w_gate.T                       [B, G]
  gate   = straight-through gumbel softmax       [B, G]  (~one-hot)
  z      = conv_w @ x + conv_b                   [B, C, L]  (pointwise conv)
  out1   = z * (gate @ MASK)                     (channel gating)
  xn     = (out1 - gate@rmean) / sqrt(gate@rvar + eps) * bn_w + bn_b
  out    = xn * (gate @ MASK)

Everything after the pool reduces to a per-(batch,channel) affine applied to
the conv output:  out[b,c,l] = z_mm[b,c,l] * S[b,c] + T[b,c]  where z_mm is
the pure matmul part and S/T fold the gate, conv bias and BN constants.

Sharding: data-parallel over batch, 2 batches per core, 8 cores.  Per core:

  phase A: stream all 32 x-chunks, per-channel sums -> pool.  RES_PER_B
           chunks per batch are DMA'd straight into persistent SBUF tiles;
           the idle PE converts them to z = conv_w @ x in place.  The other
           chunks go through a small recycling pool.
  gate(b): tiny per-batch gating chain -> S[:,b], T[:,b]
  phase C: non-resident chunks: re-stream x, z = conv_w@x (PE), epilogue
           z*S+T (ACT/DVE), store.  Resident chunks: in-place z*S+T, store
           (pure writes, scheduled to drain at the tail).

HBM traffic per core: 64 + 2*(16 - RES_PER_B) MiB  (vs 96 naive).
"""

import numpy as np

import concourse.bass as bass
import concourse.tile as tile
from concourse import mybir, bacc
from concourse.bass_utils import run_bass_kernel_spmd
from concourse.masks import make_identity

F32 = mybir.dt.float32

B, C, L, G = 16, 128, 32768, 4
NCORES = 8
BPC = B // NCORES          # batches per core
CHANNELS = [32, 64, 96, 128]
EPS = 1e-5

LC = 2048                  # columns per DMA chunk
NCHUNK = L // LC           # chunks per batch
MMN = 512                  # matmul moving-dim (fp32 max)
NMM = LC // MMN            # matmuls per chunk
N_RES = 17                 # total resident-z chunks (of BPC*NCHUNK)

AX = mybir.AxisListType.X
ALU = mybir.AluOpType
ACTF = mybir.ActivationFunctionType


def build_kernel(l_total=L, n_res=N_RES):
    nchunk = l_total // LC
    n_res = min(n_res, BPC * nchunk)
    nc = bacc.Bacc("TRN2", target_bir_lowering=False)

    x_d = nc.declare_dram_parameter("x", [BPC, C, l_total], F32, isOutput=False)
    gum_d = nc.declare_dram_parameter("gumbel", [BPC, G], F32, isOutput=False)
    wg_d = nc.declare_dram_parameter("w_gate", [G, C], F32, isOutput=False)
    cw_d = nc.declare_dram_parameter("conv_w", [C, C], F32, isOutput=False)
    cb_d = nc.declare_dram_parameter("conv_b", [C, 1], F32, isOutput=False)
    bw_d = nc.declare_dram_parameter("bn_w", [C, 1], F32, isOutput=False)
    bb_d = nc.declare_dram_parameter("bn_b", [C, 1], F32, isOutput=False)
    rm_d = nc.declare_dram_parameter("rmean", [G, C], F32, isOutput=False)
    rv_d = nc.declare_dram_parameter("rvar", [G, C], F32, isOutput=False)
    out_d = nc.declare_dram_parameter("out", [BPC, C, l_total], F32, isOutput=True)

    def is_res(b, ci):
        return b * nchunk + ci < n_res

    with tile.TileContext(nc) as tc:
        with (
            tc.tile_pool(name="consts", bufs=1) as consts,
            tc.tile_pool(name="xin", bufs=6) as xin_pool,
            tc.tile_pool(name="zres", bufs=1) as zres_pool,
            tc.tile_pool(name="small", bufs=1) as small,
            tc.tile_pool(name="psz", bufs=5, space="PSUM") as psz,
            tc.tile_pool(name="pss", bufs=1, space="PSUM") as pss,
        ):
            # ---- constants ----
            id128 = consts.tile([128, 128], F32)
            make_identity(nc, id128)

            w_oi = consts.tile([C, C], F32)
            nc.sync.dma_start(out=w_oi, in_=cw_d.ap())
            wt_ps = pss.tile([C, C], F32, tag="big")
            nc.tensor.transpose(out=wt_ps, in_=w_oi, identity=id128)
            convwT = consts.tile([C, C], F32)       # [i, o] = conv_w[o, i]
            nc.vector.tensor_copy(out=convwT, in_=wt_ps)

            wgT = consts.tile([C, G], F32)          # [c, g] = w_gate[g, c]
            nc.sync.dma_start(out=wgT, in_=wg_d.ap().rearrange("g c -> c g"))
            gum_rows = []
            for b in range(BPC):
                gr = consts.tile([1, G], F32, tag=f"gum{b}")
                nc.sync.dma_start(out=gr, in_=gum_d.ap()[b:b + 1, :])
                gum_rows.append(gr)
            cb_sb = consts.tile([C, 1], F32)
            nc.sync.dma_start(out=cb_sb, in_=cb_d.ap())
            bw_sb = consts.tile([C, 1], F32)
            nc.sync.dma_start(out=bw_sb, in_=bw_d.ap())
            bb_sb = consts.tile([C, 1], F32)
            nc.sync.dma_start(out=bb_sb, in_=bb_d.ap())
            rm_sb = consts.tile([G, C], F32)
            nc.sync.dma_start(out=rm_sb, in_=rm_d.ap())
            rv_sb = consts.tile([G, C], F32)
            nc.sync.dma_start(out=rv_sb, in_=rv_d.ap())

            eps_sb = consts.tile([C, 1], F32)
            nc.vector.memset(eps_sb, EPS)

            # MASK[g, c] = 1.0 if c < CHANNELS[g] = 32*(g+1) else 0.0
            # iota = -32 - 32*g + c ; >= 0 -> keep in_ (0), else fill (1)
            mask_sb = consts.tile([G, C], F32)
            nc.gpsimd.memset(mask_sb, 0.0)
            nc.gpsimd.affine_select(
                out=mask_sb, in_=mask_sb, compare_op=ALU.is_ge, fill=1.0,
                base=-CHANNELS[0], channel_multiplier=-CHANNELS[0],
                pattern=[[1, C]])

            partials = consts.tile([C, BPC * nchunk], F32)
            pool_sb = consts.tile([C, BPC], F32)
            S_sb = consts.tile([C, BPC], F32)
            T_sb = consts.tile([C, BPC], F32)

            # ---- precompute S/T for each of the G possible gate choices ----
            # (forward gate is numerically the hard one-hot; see module doc)
            rmT = consts.tile([C, G], F32)          # [c, g] = rmean[g, c]
            nc.sync.dma_start(out=rmT, in_=rm_d.ap().rearrange("g c -> c g"))
            rvT = consts.tile([C, G], F32)
            nc.sync.dma_start(out=rvT, in_=rv_d.ap().rearrange("g c -> c g"))
            # maskT[c, g] = 1.0 if c < 32*(g+1) else 0.0
            maskT = consts.tile([C, G], F32)
            nc.gpsimd.memset(maskT, 0.0)
            nc.gpsimd.affine_select(
                out=maskT, in_=maskT, compare_op=ALU.is_ge, fill=1.0,
                base=-CHANNELS[0], channel_multiplier=1,
                pattern=[[-CHANNELS[0], G]])

            stdA = small.tile([C, G], F32)
            nc.scalar.activation(out=stdA, in_=rvT, func=ACTF.Sqrt,
                                 bias=eps_sb, scale=1.0)
            istdA = small.tile([C, G], F32)
            nc.vector.reciprocal(out=istdA, in_=stdA)
            # S_all = mask * istd * bn_w      (mask^2 == mask)
            S_all = small.tile([C, G], F32)
            nc.vector.tensor_mul(out=S_all, in0=maskT, in1=istdA)
            nc.vector.tensor_scalar_mul(out=S_all, in0=S_all, scalar1=bw_sb)
            # T_all = ((conv_b*mask - rmean) * istd * bn_w + bn_b) * mask
            T_all = small.tile([C, G], F32)
            nc.vector.tensor_scalar_mul(out=T_all, in0=maskT, scalar1=cb_sb)
            nc.vector.tensor_sub(out=T_all, in0=T_all, in1=rmT)
            nc.vector.tensor_mul(out=T_all, in0=T_all, in1=istdA)
            nc.vector.tensor_scalar(out=T_all, in0=T_all, scalar1=bw_sb,
                                    scalar2=bb_sb, op0=ALU.mult, op1=ALU.add)
            nc.vector.tensor_mul(out=T_all, in0=T_all, in1=maskT)
            # transpose to [G, C] for the one-hot selection matmuls
            sat_ps = pss.tile([G, C], F32, tag="big")
            nc.tensor.transpose(out=sat_ps, in_=S_all, identity=id128)
            S_allT = consts.tile([G, C], F32)
            nc.vector.tensor_copy(out=S_allT, in_=sat_ps)
            tat_ps = pss.tile([G, C], F32, tag="big")
            nc.tensor.transpose(out=tat_ps, in_=T_all, identity=id128)
            T_allT = consts.tile([G, C], F32)
            nc.vector.tensor_copy(out=T_allT, in_=tat_ps)

            zres = {}

            def phase_a_chunk(b, ci):
                col = b * nchunk + ci
                if is_res(b, ci):
                    dst = zres_pool.tile([C, LC], F32, tag=f"zres{col}")
                    zres[(b, ci)] = dst
                else:
                    dst = xin_pool.tile([C, LC], F32, tag="xin")
                nc.sync.dma_start(
                    out=dst, in_=x_d.ap()[b, :, ci * LC:(ci + 1) * LC])
                nc.vector.reduce_sum(
                    out=partials[:, col:col + 1], in_=dst, axis=AX)
                if is_res(b, ci):
                    # convert x -> z in place (PE matmul + ACT drain)
                    for j in range(NMM):
                        js = slice(j * MMN, (j + 1) * MMN)
                        zp = psz.tile([C, MMN], F32)
                        nc.tensor.matmul(out=zp, lhsT=convwT, rhs=dst[:, js],
                                         start=True, stop=True)
                        nc.scalar.copy(out=dst[:, js], in_=zp)

            def finish_pool(b):
                nc.vector.reduce_sum(
                    out=pool_sb[:, b:b + 1],
                    in_=partials[:, b * nchunk:(b + 1) * nchunk], axis=AX)
                nc.scalar.mul(out=pool_sb[:, b:b + 1],
                              in_=pool_sb[:, b:b + 1], mul=1.0 / l_total)

            def gate_phase(b):
                """Short gating chain: logits -> hard one-hot -> select
                precomputed S/T columns via tiny matmuls."""
                lg_ps = pss.tile([1, G], F32, tag="lg")
                nc.tensor.matmul(out=lg_ps, lhsT=pool_sb[:, b:b + 1], rhs=wgT,
                                 start=True, stop=True)
                y_sb = small.tile([1, G], F32, tag=f"y{b}")
                nc.vector.tensor_add(out=y_sb, in0=lg_ps, in1=gum_rows[b])
                m1 = small.tile([1, 1], F32, tag=f"m1{b}")
                nc.vector.reduce_max(out=m1, in_=y_sb, axis=AX)
                yhard = small.tile([1, G], F32, tag=f"yh{b}")
                nc.vector.tensor_scalar(out=yhard, in0=y_sb, scalar1=m1,
                                        scalar2=None, op0=ALU.is_ge)
                gt_ps = pss.tile([G, 1], F32, tag="gt")
                nc.tensor.transpose(out=gt_ps, in_=yhard,
                                    identity=id128[0:1, 0:1])
                gateT = small.tile([G, 1], F32, tag=f"gT{b}")
                nc.vector.tensor_copy(out=gateT, in_=gt_ps)

                sel_ps = pss.tile([C, 2], F32, tag="big")
                nc.tensor.matmul(out=sel_ps[:, 0:1], lhsT=S_allT, rhs=gateT,
                                 start=True, stop=True)
                nc.tensor.matmul(out=sel_ps[:, 1:2], lhsT=T_allT, rhs=gateT,
                                 start=True, stop=True)
                nc.vector.tensor_copy(out=S_sb[:, b:b + 1], in_=sel_ps[:, 0:1])
                nc.vector.tensor_copy(out=T_sb[:, b:b + 1], in_=sel_ps[:, 1:2])

            def c_chunk(b, ci, epi_parity):
                S_col = S_sb[:, b:b + 1]
                T_col = T_sb[:, b:b + 1]
                sl = slice(ci * LC, (ci + 1) * LC)
                if is_res(b, ci):
                    zt = zres[(b, ci)]
                    nc.vector.tensor_scalar(
                        out=zt, in0=zt, scalar1=S_col, scalar2=T_col,
                        op0=ALU.mult, op1=ALU.add)
                    nc.sync.dma_start(out=out_d.ap()[b, :, sl], in_=zt)
                    return
                xc = xin_pool.tile([C, LC], F32, tag="xin")
                nc.sync.dma_start(out=xc, in_=x_d.ap()[b, :, sl])
                for j in range(NMM):
                    js = slice(j * MMN, (j + 1) * MMN)
                    zp = psz.tile([C, MMN], F32)
                    nc.tensor.matmul(out=zp, lhsT=convwT, rhs=xc[:, js],
                                     start=True, stop=True)
                    if (epi_parity * NMM + j) % 2 == 0:
                        nc.scalar.activation(out=xc[:, js], in_=zp,
                                             func=ACTF.Identity,
                                             bias=T_col, scale=S_col)
                    else:
                        nc.vector.tensor_scalar(
                            out=xc[:, js], in0=zp, scalar1=S_col,
                            scalar2=T_col, op0=ALU.mult, op1=ALU.add)
                nc.sync.dma_start(out=out_d.ap()[b, :, sl], in_=xc)

            # ---- emission order ----
            with nc.named_scope("phaseA"):
                for b in range(BPC):
                    for ci in range(nchunk):
                        phase_a_chunk(b, ci)
                    finish_pool(b)
            with nc.named_scope("gates"):
                for b in range(BPC):
                    gate_phase(b)
            # phase C: non-resident first (reads can prefetch during the
            # gating chain), resident last (pure writes drain the tail).
            with nc.named_scope("phaseC"):
                parity = 0
                for b in range(BPC):
                    for ci in range(nchunk):
                        if not is_res(b, ci):
                            c_chunk(b, ci, parity)
                            parity += 1
                for b in range(BPC):
                    for ci in range(nchunk):
                        if is_res(b, ci):
                            c_chunk(b, ci, 0)

    nc.compile()
    return nc


_NC = None


def _get_nc():
    global _NC
    if _NC is None:
        _NC = build_kernel()
    return _NC


def kernel(x, gumbel_noise, w_gate, conv_w, conv_b, bn_w, bn_b, rmean, rvar):
    nc = _get_nc()
    f = lambda a: np.ascontiguousarray(a, dtype=np.float32)
    shared = {
        "w_gate": f(w_gate),
        "conv_w": f(conv_w),
        "conv_b": f(conv_b).reshape(C, 1),
        "bn_w": f(bn_w).reshape(C, 1),
        "bn_b": f(bn_b).reshape(C, 1),
        "rmean": f(rmean),
        "rvar": f(rvar),
    }
    in_maps = []
    for i in range(NCORES):
        sl = slice(i * BPC, (i + 1) * BPC)
        in_maps.append({"x": f(x[sl]), "gumbel": f(gumbel_noise[sl]), **shared})
    res = run_bass_kernel_spmd(nc, in_maps, list(range(NCORES)))
    out = np.concatenate([res.results[i]["out"] for i in range(NCORES)], axis=0)
    return out.astype(np.float32, copy=False)
